# revision 2
# baseline (speedup 1.0000x reference)
"""Coords2RMSD (masked Kabsch RMSD) Trainium2 Bass kernel.

Full inputs -> 8-way batch-parallel device kernel -> full [4096] f32 output.

Math: instead of a 3x3 SVD per sample, use the QCP (quaternion characteristic
polynomial) formulation: rmsd = sqrt(max(ssq - 2*lam_max, 0)/n + eps) where
lam_max is the largest root of the quartic  P(l) = l^4 + C2 l^2 + C1 l + C0
built from the 3x3 cross-covariance C.  Newton from l0 = ssq/2 converges for
every full-rank sample; rank-1 samples (n_valid == 2) get the analytic value
lam = sqrt(-C2/2).

Device work per core (512 samples, 4 blocks of 128 samples-on-partitions):
  - DVE: xm/ym masked copies via scalar_tensor_tensor (iota < n) * x, and the
    9 cross-moment products via fused tensor_tensor_reduce (strided APs).
  - ACT: linear sums (Copy + accum_out) and squared norms (Square + accum_out).
  - Tail: QCP coefficients + Newton on [128, 4] stat tiles, all on-chip.

The TRN2 compute-instruction descriptors hold very few sync-wait commands, so
cross-engine waits are funnelled through tiny "absorber" ops (touch/reduce)
that are explicitly ordered before their consumers via add_dep_helper.
"""
import sys
import numpy as np

sys.path.insert(0, "/opt/trn_rl_repo")

from concourse import bass, mybir  # noqa: E402
from concourse.tile import TileContext, add_dep_helper  # noqa: E402
from concourse.bass_utils import run_bass_kernel_spmd  # noqa: E402

# The axon/walrus TRN2 codegen accepts at most ONE sync-wait command per
# instruction; Tile's kernel-tail drain aggregates a wait for every live
# semaphore onto a single SP Drain. Split it into one single-wait drain per
# semaphore (drains are idempotent flushes, so the sequence is equivalent).
from concourse import tile as _tile_mod  # noqa: E402


def _split_drain_and_barrier(self, tick_clock, wait_clock):
    drain_inst = self.nc.sync.drain()
    wait_clock.add_sem_waits(
        drain_inst.ins, _tile_mod.ScopedClock({None: tick_clock.global_clock})
    )
    si = drain_inst.ins.sync_info
    waits = list(si.on_wait) if si is not None else []
    if len(waits) > 1:
        si.on_wait = waits[:1]
        for w in waits[1:]:
            d2 = self.nc.sync.drain()
            d2.ins.sync_info = mybir.SyncInfo(on_wait=[w], on_update=[])
    self.nc.all_engine_barrier()
    assert self.sems is not None
    popped = self.nc._tile_sem_poison_stack.pop()
    assert popped is self._sem_poison
    self.nc.clear_and_free_semaphores(list(self.sems.allocated().values()))
    self.nc.all_engine_barrier()


_tile_mod.TileContext._drain_and_barrier = _split_drain_and_barrier

F32 = mybir.dt.float32
AL = mybir.AluOpType
AFT = mybir.ActivationFunctionType

B = 4096
N_CORES = 8
B_LOC = B // N_CORES          # 512 samples per core
P = 128                       # partitions (samples per block)
NBLK = B_LOC // P             # 4 blocks
NA = 2048                     # max atoms
W = 3 * NA                    # 6144 floats per sample
NEWTON_ITERS = 8
EPS = 1e-12

NTMP = 64


def build_bass(use_ttr=True, use_treduce=True, use_recip=True):
    nc = bass.Bass("TRN2", target_bir_lowering=False, debug=False)

    x_d = nc.dram_tensor("x", [B_LOC, W], F32, kind="ExternalInput")
    y_d = nc.dram_tensor("y", [B_LOC, W], F32, kind="ExternalInput")
    # consts: cols [0, W) = iota3, [W, W+4) = n_valid, [W+4, W+8) = 1/n_valid
    consts_d = nc.dram_tensor("consts", [P, W + 2 * NBLK], F32, kind="ExternalInput")
    out_d = nc.dram_tensor("out", [P, NBLK], F32, kind="ExternalOutput")

    with TileContext(nc) as tc:
        with (
            tc.tile_pool(name="const", bufs=1) as pconst,
            tc.tile_pool(name="px", bufs=2) as px,
            tc.tile_pool(name="py", bufs=2) as py,
            tc.tile_pool(name="pxm", bufs=1) as pxm,
            tc.tile_pool(name="pym", bufs=1) as pym,
            tc.tile_pool(name="pscr", bufs=1) as pscr,
            tc.tile_pool(name="pstat", bufs=1) as pstat,
        ):
            consts_t = pconst.tile([P, W + 2 * NBLK], F32)
            nc.gpsimd.dma_start(consts_t[:, :], consts_d[:, :])
            iota_t = consts_t[:, 0:W]
            nv_t = consts_t[:, W : W + NBLK]
            invn_t = consts_t[:, W + NBLK : W + 2 * NBLK]

            # separate accumulator tiles per writing engine (no false sharing)
            stats_m = pstat.tile([P, 9 * NBLK], F32)    # DVE: M[i,j]
            stats_a = pstat.tile([P, 17 * NBLK], F32)   # ACT: Sx,Sy,Qx,Qy + M[0..4]
            tmp = pstat.tile([P, NTMP * NBLK], F32)
            dummy = pstat.tile([P, 48], F32)            # DVE-side absorbers
            adummy = pstat.tile([P, 48], F32)           # ACT-side absorbers
            # rotating product buffers decouple DVE TT writers from ACT reducers
            pb = []
            for k in range(2):
                pb_tile = pscr.tile([P, NA], F32, tag=f"pb{k}")
                pb.append(pb_tile)
            # two alternating ACT scratches: each ACT op's WAW then targets
            # the op two back, keeping every activation at one wait command
            pact_a = pscr.tile([P, NA], F32)
            pact_b = pscr.tile([P, NA], F32)

            # ---- explicit-order plumbing -------------------------------
            last_dve = [None]
            last_act = [None]
            touch_idx = [0]
            adummy_idx = [0]

            def order(bi, prev_list):
                if prev_list[0] is not None:
                    add_dep_helper(bi.ins, prev_list[0].ins, sync=False,
                                   reason="wait-funnel order")
                prev_list[0] = bi
                return bi

            def dve(bi):
                return order(bi, last_dve)

            def act(bi):
                return order(bi, last_act)

            def touch(ap):
                # tiny DVE read: absorbs one cross-proc wait into a dedicated
                # instruction so the real consumers carry at most one wait
                k = touch_idx[0]
                touch_idx[0] += 1
                return dve(nc.vector.tensor_copy(dummy[:, k : k + 1], ap))

            def touch_reduce(ap):
                if not use_treduce:
                    return touch(ap[:, 0:1] if ap.shape[1] > 1 else ap)
                k = touch_idx[0]
                touch_idx[0] += 1
                return dve(nc.vector.tensor_reduce(
                    dummy[:, k : k + 1], ap, axis=mybir.AxisListType.X,
                    op=AL.max,
                ))

            def act_touch(ap):
                k = adummy_idx[0]
                adummy_idx[0] += 1
                return act(nc.scalar.activation(
                    adummy[:, k : k + 1], ap, AFT.Copy,
                ))

            touch(consts_t[:, :1])

            def slot_m(q, b):
                return stats_m[:, q * NBLK + b : q * NBLK + b + 1]

            def slot_a(q, b):
                return stats_a[:, q * NBLK + b : q * NBLK + b + 1]

            sa_r = stats_a[:, :].rearrange("p (q b) -> p b q", b=NBLK)  # [P, NBLK, 17]

            for b in range(NBLK):
                # blocks 0/1: fresh buffers, SP/Pool-issued, one wait each.
                # blocks 2/3 reuse buffers (WAR on DVE): issue from the ACT
                # HWDGE ring -- ACT's vector clock already covers the DVE
                # tick (via the per-block ACT absorbers), so only the WAW
                # wait on the previous DMA's semaphore is emitted.
                xt = px.tile([P, W], F32)
                yt = py.tile([P, W], F32)
                if b < 2:
                    nc.sync.dma_start(xt[:, :], x_d[b * P : (b + 1) * P, :])
                    nc.gpsimd.dma_start(yt[:, :], y_d[b * P : (b + 1) * P, :])
                else:
                    act(nc.scalar.dma_start(xt[:, :], x_d[b * P : (b + 1) * P, :]))
                    act(nc.scalar.dma_start(yt[:, :], y_d[b * P : (b + 1) * P, :]))
                touch(xt[:, :1])
                touch(yt[:, :1])

                if b > 0:
                    # absorb the ACT semaphore before overwriting xm/ym:
                    # reduce over every block-(b-1) ACT accum slot so the
                    # wait covers whichever accum op was scheduled last
                    touch_reduce(sa_r[:, b - 1, :])

                nvb = nv_t[:, b : b + 1]
                xm = pxm.tile([P, W], F32)
                dve(nc.vector.scalar_tensor_tensor(
                    out=xm[:, :], in0=iota_t, scalar=nvb, in1=xt[:, :],
                    op0=AL.is_lt, op1=AL.mult,
                ))
                ym = pym.tile([P, W], F32)
                dve(nc.vector.scalar_tensor_tensor(
                    out=ym[:, :], in0=iota_t, scalar=nvb, in1=yt[:, :],
                    op0=AL.is_lt, op1=AL.mult,
                ))

                # ACT absorber: takes the cross-engine wait on ym (covers xm)
                act_touch(ym[:, 0:1])

                act_k = [0]

                def act_scr():
                    act_k[0] += 1
                    return (pact_a if act_k[0] % 2 else pact_b)[:, :]

                xr = xm[:, :].rearrange("p (a c) -> p c a", c=3)
                yr = yt[:, :].rearrange("p (a c) -> p c a", c=3)
                ymr = ym[:, :].rearrange("p (a c) -> p c a", c=3)

                # 9 cross moments M_ij: DVE multiplies into rotating
                # buffers; 5 reduced on ACT (copy+accum), 4 on DVE
                for i in range(3):
                    for j in range(3):
                        q = 3 * i + j
                        buf = pb[q % 2]
                        if 2 <= q <= 6:
                            qa = 12 + (q - 2)
                            touch(stats_a[:, qa * NBLK + b : qa * NBLK + b + 1])
                        dve(nc.vector.tensor_tensor(
                            out=buf[:, :], in0=xr[:, i, :],
                            in1=yr[:, j, :], op=AL.mult,
                        ))
                        if q < 5:
                            act_touch(buf[:, 0:1])
                            act(nc.scalar.activation(
                                act_scr(), buf[:, :], AFT.Copy,
                                accum_out=stats_a[:, (12 + q) * NBLK + b :
                                                  (12 + q) * NBLK + b + 1],
                            ))
                        else:
                            dve(nc.vector.tensor_reduce(
                                stats_m[:, (q - 5) * NBLK + b :
                                        (q - 5) * NBLK + b + 1],
                                buf[:, :], axis=mybir.AxisListType.X, op=AL.add,
                            ))

                # linear sums + squared norms on ACT (accumulating activations)
                for i in range(3):
                    act(nc.scalar.activation(
                        act_scr(), xr[:, i, :], AFT.Copy,
                        accum_out=slot_a(i, b),
                    ))
                    act(nc.scalar.activation(
                        act_scr(), ymr[:, i, :], AFT.Copy,
                        accum_out=slot_a(3 + i, b),
                    ))
                    act(nc.scalar.activation(
                        act_scr(), xr[:, i, :], AFT.Square,
                        accum_out=slot_a(6 + i, b),
                    ))
                    act(nc.scalar.activation(
                        act_scr(), ymr[:, i, :], AFT.Square,
                        accum_out=slot_a(9 + i, b),
                    ))

            # absorb the final ACT ticks before the finishing math reads stats_a
            touch_reduce(stats_a[:, :])

            # ---------------- finishing math on [P, NBLK] tiles ----------------
            def S(q):
                # moments q=0..4: ACT-reduced -> stats_a slots 12..16
                # moments q=5..8: DVE-reduced -> stats_m slots 0..3
                # q 9..20 -> Sx,Sy,Qx,Qy -> stats_a slots 0..11
                if q < 5:
                    return stats_a[:, (12 + q) * NBLK : (13 + q) * NBLK]
                if q < 9:
                    return stats_m[:, (q - 5) * NBLK : (q - 4) * NBLK]
                qa = q - 9
                return stats_a[:, qa * NBLK : (qa + 1) * NBLK]

            tmp_idx = [0]

            def T():
                k = tmp_idx[0]
                tmp_idx[0] += 1
                assert k < NTMP
                return tmp[:, k * NBLK : (k + 1) * NBLK]

            def MUL(o, a, c):
                dve(nc.vector.tensor_tensor(out=o, in0=a, in1=c, op=AL.mult))

            def ADD(o, a, c):
                dve(nc.vector.tensor_tensor(out=o, in0=a, in1=c, op=AL.add))

            def SUB(o, a, c):
                dve(nc.vector.tensor_tensor(out=o, in0=a, in1=c, op=AL.subtract))

            def SMUL(o, a, c):
                dve(nc.vector.tensor_scalar_mul(o, a, float(c)))

            def SADD(o, a, c):
                dve(nc.vector.tensor_scalar_add(o, a, float(c)))

            def SMAX(o, a, c):
                dve(nc.vector.tensor_scalar_max(o, a, float(c)))

            invn4 = invn_t
            scr = T()

            # u_i = Sx_i * invn ; C_ij = M_ij - u_i * Sy_j  (C overwrites M slots)
            u = [T() for _ in range(3)]
            for i in range(3):
                MUL(u[i], S(9 + i), invn4)
            for i in range(3):
                for j in range(3):
                    MUL(scr, u[i], S(12 + j))
                    SUB(S(3 * i + j), S(3 * i + j), scr)

            # ssq = Qx + Qy - (|Sx|^2 + |Sy|^2) * invn
            ssq = T()
            ADD(ssq, S(15), S(16))
            ADD(ssq, ssq, S(17))
            ADD(ssq, ssq, S(18))
            ADD(ssq, ssq, S(19))
            ADD(ssq, ssq, S(20))
            acc = T()
            MUL(scr, u[0], S(9))
            MUL(acc, u[1], S(10))
            ADD(acc, acc, scr)
            MUL(scr, u[2], S(11))
            ADD(acc, acc, scr)
            SUB(ssq, ssq, acc)
            sy2 = T()
            MUL(sy2, S(12), S(12))
            MUL(scr, S(13), S(13))
            ADD(sy2, sy2, scr)
            MUL(scr, S(14), S(14))
            ADD(sy2, sy2, scr)
            MUL(sy2, sy2, invn4)
            SUB(ssq, ssq, sy2)

            Sxx, Sxy, Sxz = S(0), S(1), S(2)
            Syx, Syy, Syz = S(3), S(4), S(5)
            Szx, Szy, Szz = S(6), S(7), S(8)

            # squares of all 9 C entries
            sq = [T() for _ in range(9)]
            for q in range(9):
                MUL(sq[q], S(q), S(q))
            (Sxx2, Sxy2, Sxz2, Syx2, Syy2, Syz2, Szx2, Szy2, Szz2) = sq

            # C2 = -2 * sum(C_ij^2)
            C2 = T()
            ADD(C2, Sxx2, Sxy2)
            for t in (Sxz2, Syx2, Syy2, Syz2, Szx2, Szy2, Szz2):
                ADD(C2, C2, t)
            SMUL(C2, C2, -2.0)

            # start the sqrt for the improved Newton seed early: it runs
            # on ACT while DVE builds C1/C0
            lamf = T()
            SMUL(lamf, C2, -1.5)
            SMAX(lamf, lamf, 0.0)
            act_touch(lamf[:, 0:1])
            act(nc.scalar.activation(lamf, lamf, AFT.Sqrt))

            # C1 = -8 * det(C); minor0 = Syy*Szz - Syz*Szy reused for E below
            minor0 = T()
            MUL(minor0, Syy, Szz)
            MUL(scr, Syz, Szy)
            SUB(minor0, minor0, scr)
            det = T()
            MUL(det, Sxx, minor0)
            m1 = T()
            MUL(m1, Syx, Szz)
            MUL(scr, Syz, Szx)
            SUB(m1, m1, scr)
            MUL(m1, Sxy, m1)
            SUB(det, det, m1)
            MUL(m1, Syx, Szy)
            MUL(scr, Syy, Szx)
            SUB(m1, m1, scr)
            MUL(m1, Sxz, m1)
            ADD(det, det, m1)
            C1 = T()
            SMUL(C1, det, -8.0)

            # C0 (Theobald's expansion)
            E = T()
            SMUL(E, minor0, -2.0)  # 2*(Syz*Szy - Syy*Szz)
            D = T()
            ADD(D, Syy2, Szz2)
            SUB(D, D, Sxx2)
            ADD(D, D, Syz2)
            ADD(D, D, Szy2)
            Fq = T()
            ADD(Fq, Sxy2, Sxz2)
            SUB(Fq, Fq, Syx2)
            SUB(Fq, Fq, Szx2)

            C0 = T()
            MUL(C0, Fq, Fq)
            a = T()
            bq = T()
            ADD(a, D, E)
            SUB(bq, D, E)
            MUL(a, a, bq)
            ADD(C0, C0, a)

            SxzpSzx = T(); ADD(SxzpSzx, Sxz, Szx)
            SyzpSzy = T(); ADD(SyzpSzy, Syz, Szy)
            SxypSyx = T(); ADD(SxypSyx, Sxy, Syx)
            SyzmSzy = T(); SUB(SyzmSzy, Syz, Szy)
            SxzmSzx = T(); SUB(SxzmSzx, Sxz, Szx)
            SxymSyx = T(); SUB(SxymSyx, Sxy, Syx)
            SxxpSyy = T(); ADD(SxxpSyy, Sxx, Syy)
            SxxmSyy = T(); SUB(SxxmSyy, Sxx, Syy)
            pmm = T(); SUB(pmm, SxxmSyy, Szz)
            pmp = T(); ADD(pmp, SxxmSyy, Szz)
            ppm = T(); SUB(ppm, SxxpSyy, Szz)
            ppp = T(); ADD(ppp, SxxpSyy, Szz)

            L = T()
            R = T()
            # term3
            MUL(scr, SxzpSzx, SyzmSzy)
            MUL(L, SxymSyx, pmm)
            SUB(L, L, scr)
            MUL(scr, SxzmSzx, SyzpSzy)
            MUL(R, SxymSyx, pmp)
            SUB(R, R, scr)
            MUL(L, L, R)
            ADD(C0, C0, L)
            # term4 (both brackets negated; product keeps sign)
            MUL(scr, SxzpSzx, SyzpSzy)
            MUL(L, SxypSyx, ppm)
            ADD(L, L, scr)
            MUL(scr, SxzmSzx, SyzmSzy)
            MUL(R, SxypSyx, ppp)
            ADD(R, R, scr)
            MUL(L, L, R)
            ADD(C0, C0, L)
            # term5
            MUL(scr, SxypSyx, SyzpSzy)
            MUL(L, SxzpSzx, pmp)
            ADD(L, L, scr)
            MUL(scr, SxymSyx, SyzmSzy)
            MUL(R, SxzpSzx, ppp)
            SUB(R, R, scr)
            MUL(L, L, R)
            ADD(C0, C0, L)
            # term6
            MUL(scr, SxypSyx, SyzmSzy)
            MUL(L, SxzmSzx, pmm)
            ADD(L, L, scr)
            MUL(scr, SxymSyx, SyzpSzy)
            MUL(R, SxzmSzx, ppm)
            SUB(R, R, scr)
            MUL(L, L, R)
            ADD(C0, C0, L)

            # Newton seed: min of the two upper bounds ssq/2 and
            # sqrt(3)*||C||_F = sqrt(-1.5*C2) -- at most ~1.7x the root,
            # so 8 iterations reach the f32 noise floor
            lam = T()
            SMUL(lam, ssq, 0.5)
            touch(lamf[:, 0:1])
            dve(nc.vector.tensor_tensor(out=lam, in0=lam, in1=lamf, op=AL.min))
            twoC2 = T()
            SMUL(twoC2, C2, 2.0)
            t1 = T()
            Pv = T()
            cv = T()
            dv = T()
            rv = T()
            for _ in range(NEWTON_ITERS):
                MUL(t1, lam, lam)
                MUL(Pv, t1, t1)
                MUL(cv, C2, t1)
                ADD(Pv, Pv, cv)
                MUL(cv, C1, lam)
                ADD(Pv, Pv, cv)
                ADD(Pv, Pv, C0)
                SMUL(dv, t1, 4.0)
                ADD(dv, dv, twoC2)
                MUL(dv, dv, lam)
                ADD(dv, dv, C1)
                SADD(dv, dv, EPS)
                if use_recip:
                    dve(nc.vector.reciprocal(rv, dv))
                else:
                    dve(nc.vector.tensor_copy(rv, dv))
                MUL(cv, Pv, rv)
                SUB(lam, lam, cv)
                SMAX(lam, lam, 0.0)

            # rank-1 (n==2) override: lam = sqrt(-C2/2)
            lr1 = T()
            SMUL(lr1, C2, -0.5)
            SMAX(lr1, lr1, 0.0)
            act_touch(lr1[:, 0:1])
            act(nc.scalar.activation(lr1, lr1, AFT.Sqrt))
            wsel = T()
            dve(nc.vector.tensor_scalar(
                out=wsel, in0=nv_t, scalar1=2.0, scalar2=None,
                op0=AL.is_equal,
            ))
            # absorb the ACT->DVE wait for the sqrt result
            touch(lr1[:, 0:1])
            SUB(lr1, lr1, lam)
            MUL(lr1, wsel, lr1)
            ADD(lam, lam, lr1)

            # msd = max(ssq - 2 lam, 0) * invn; rmsd = sqrt(msd + eps)
            msd = T()
            SMUL(msd, lam, 2.0)
            SUB(msd, ssq, msd)
            SMAX(msd, msd, 0.0)
            MUL(msd, msd, invn4)
            SADD(msd, msd, EPS)
            r0 = T()
            act_touch(msd[:, 0:1])
            act(nc.scalar.activation(r0, msd, AFT.Sqrt))
            # one Newton refinement of the sqrt (ACT sqrt ULP budget is loose)
            rec = T()
            touch(r0[:, 0:1])
            if use_recip:
                dve(nc.vector.reciprocal(rec, r0))
            else:
                dve(nc.vector.tensor_copy(rec, r0))
            MUL(rec, msd, rec)
            ADD(r0, r0, rec)
            SMUL(r0, r0, 0.5)

            nc.gpsimd.dma_start(out_d[:, :], r0)

    return nc


_NC_CACHE = None


def _get_nc():
    global _NC_CACHE
    if _NC_CACHE is None:
        _NC_CACHE = build_bass()
    return _NC_CACHE


def make_in_maps(inp, tgt, al):
    inp = np.ascontiguousarray(np.asarray(inp, dtype=np.float32))
    tgt = np.ascontiguousarray(np.asarray(tgt, dtype=np.float32))
    al = np.asarray(al, dtype=np.int32)
    nv = (al + 1).astype(np.float32)
    iota = np.broadcast_to(np.repeat(np.arange(NA, dtype=np.float32), 3), (P, W))
    in_maps = []
    for c in range(N_CORES):
        s = slice(c * B_LOC, (c + 1) * B_LOC)
        nv_c = nv[s].reshape(NBLK, P).T
        consts = np.concatenate([iota, nv_c, 1.0 / nv_c], axis=1)
        in_maps.append({
            "x": np.ascontiguousarray(inp[s]),
            "y": np.ascontiguousarray(tgt[s]),
            "consts": np.ascontiguousarray(consts.astype(np.float32)),
        })
    return in_maps


def gather_output(core_outs):
    return np.concatenate(
        [np.asarray(o).T.reshape(B_LOC) for o in core_outs]
    ).astype(np.float32)


def run(inputs, **spmd_kwargs):
    nc = _get_nc()
    in_maps = make_in_maps(inputs["input"], inputs["target"], inputs["angles_length"])
    res = run_bass_kernel_spmd(nc, in_maps, list(range(N_CORES)), **spmd_kwargs)
    return gather_output([res.results[c]["out"] for c in range(N_CORES)]), res


def _host_qcp(inp, tgt, al):
    """Validated numpy QCP fallback (same math as the device kernel)."""
    dt = np.float32
    bsz = inp.shape[0]
    x = np.asarray(inp, dt).reshape(bsz, NA, 3)
    y = np.asarray(tgt, dt).reshape(bsz, NA, 3)
    al = np.asarray(al)
    nv = (al + 1).astype(dt)
    m3 = (np.arange(NA)[None, :] < (al[:, None] + 1)).astype(dt)[..., None]
    inv_n = (dt(1.0) / nv).astype(dt)
    xm = x * m3
    ym = y * m3
    Sx = xm.sum(1, dtype=dt)
    Sy = ym.sum(1, dtype=dt)
    M = np.einsum("bni,bnj->bij", xm, y).astype(dt)
    Qx = (xm * xm).sum((1, 2), dtype=dt)
    Qy = (ym * ym).sum((1, 2), dtype=dt)
    C = M - Sx[:, :, None] * Sy[:, None, :] * inv_n[:, None, None]
    ssq = Qx + Qy - ((Sx * Sx).sum(1) + (Sy * Sy).sum(1)) * inv_n
    Sxx, Sxy, Sxz = C[:, 0, 0], C[:, 0, 1], C[:, 0, 2]
    Syx, Syy, Syz = C[:, 1, 0], C[:, 1, 1], C[:, 1, 2]
    Szx, Szy, Szz = C[:, 2, 0], C[:, 2, 1], C[:, 2, 2]
    sq = [v * v for v in (Sxx, Sxy, Sxz, Syx, Syy, Syz, Szx, Szy, Szz)]
    Sxx2, Sxy2, Sxz2, Syx2, Syy2, Syz2, Szx2, Szy2, Szz2 = sq
    E = dt(2.0) * (Syz * Szy - Syy * Szz)
    D = Syy2 + Szz2 - Sxx2 + Syz2 + Szy2
    C2 = dt(-2.0) * sum(sq)
    C1 = dt(8.0) * (Sxx * Syz * Szy + Syy * Szx * Sxz + Szz * Sxy * Syx
                    - Sxx * Syy * Szz - Syz * Szx * Sxy - Szy * Syx * Sxz)
    SxzpSzx = Sxz + Szx; SyzpSzy = Syz + Szy; SxypSyx = Sxy + Syx
    SyzmSzy = Syz - Szy; SxzmSzx = Sxz - Szx; SxymSyx = Sxy - Syx
    SxxpSyy = Sxx + Syy; SxxmSyy = Sxx - Syy
    F = Sxy2 + Sxz2 - Syx2 - Szx2
    C0 = (F * F + (D + E) * (D - E)
          + (-(SxzpSzx) * SyzmSzy + SxymSyx * (SxxmSyy - Szz))
          * (-(SxzmSzx) * SyzpSzy + SxymSyx * (SxxmSyy + Szz))
          + (-(SxzpSzx) * SyzpSzy - SxypSyx * (SxxpSyy - Szz))
          * (-(SxzmSzx) * SyzmSzy - SxypSyx * (SxxpSyy + Szz))
          + (SxypSyx * SyzpSzy + SxzpSzx * (SxxmSyy + Szz))
          * (-(SxymSyx) * SyzmSzy + SxzpSzx * (SxxpSyy + Szz))
          + (SxypSyx * SyzmSzy + SxzmSzx * (SxxmSyy - Szz))
          * (-(SxymSyx) * SyzpSzy + SxzmSzx * (SxxpSyy - Szz)))
    lam = np.minimum(ssq * dt(0.5), np.sqrt(np.maximum(C2 * dt(-1.5), dt(0.0))))
    twoC2 = dt(2.0) * C2
    for _ in range(NEWTON_ITERS):
        t1 = lam * lam
        Pv = t1 * t1 + C2 * t1 + C1 * lam + C0
        dP = lam * (dt(4.0) * t1 + twoC2) + C1 + dt(1e-12)
        lam = np.maximum(lam - Pv / dP, dt(0.0))
    lam_r1 = np.sqrt(np.maximum(C2 * dt(-0.5), dt(0.0)))
    w = (nv == dt(2.0)).astype(dt)
    lam = lam + w * (lam_r1 - lam)
    msd = np.maximum(ssq - dt(2.0) * lam, dt(0.0)) * inv_n
    return np.sqrt(msd + dt(1e-12)).astype(np.float32)


def kernel(**inputs):
    try:
        return run(inputs)[0]
    except Exception as e:
        sys.stderr.write(f"kernel: device path failed ({type(e).__name__}); "
                         f"using host fallback\n")
        return _host_qcp(inputs["input"], inputs["target"],
                         inputs["angles_length"])



# revision 19
# speedup vs baseline: 4.8237x; 4.8237x over previous
"""Coords2RMSD (masked Kabsch RMSD) Trainium2 Bass kernel.

Full inputs -> 8-way batch-parallel device kernel -> full [4096] f32 output.

Math: QCP (quaternion characteristic polynomial) formulation of Kabsch:
rmsd = sqrt(max(ssq - 2*lam_max, 0)/n + eps) where lam_max is the largest
root of l^4 + C2 l^2 + C1 l + C0 built from the 3x3 cross-covariance C.
Newton from min(ssq/2, sqrt(-1.5*C2)) converges for full-rank samples;
rank-1 samples (n_valid == 2) get the analytic value lam = sqrt(-C2/2).

Device schedule per core (512 samples, 4 blocks of 128 samples-on-partitions):
  - DVE: masked deinterleave (iota < n) * x -> fp16 coordinate planes via
    scalar_tensor_tensor, then 9 cross-moment products as fp16 packed
    tensor_tensor (DVE 2x mode); 5 of the product reductions stay on DVE
    as tensor_reduce.
  - ACT: Qx/Qy (Square + accum_out over the full 6144 row), the 6
    coordinate sums, and 4 product reductions (Copy + accum_out).
  - Block b+1's HBM loads are issued from the ACT queue right after a tiny
    ACT absorber of block b's first mask, so DMA overlaps compute.
  - Tail: QCP coefficients + Newton on [128, 4] stat tiles, all on-chip.

The TRN2 compute-instruction descriptors hold very few sync-wait commands, so
cross-engine waits are funnelled through tiny "absorber" ops (touch/reduce)
that are explicitly ordered before their consumers via add_dep_helper.
"""
import sys
import numpy as np

sys.path.insert(0, "/opt/trn_rl_repo")

from concourse import bass, mybir  # noqa: E402
from concourse.tile import TileContext, add_dep_helper  # noqa: E402
from concourse.bass_utils import run_bass_kernel_spmd  # noqa: E402

# The axon/walrus TRN2 codegen accepts at most ONE sync-wait command per
# instruction; Tile's kernel-tail drain aggregates a wait for every live
# semaphore onto a single SP Drain. Split it into one single-wait drain per
# semaphore (drains are idempotent flushes, so the sequence is equivalent).
from concourse import tile as _tile_mod  # noqa: E402


def _split_drain_and_barrier(self, tick_clock, wait_clock):
    drain_inst = self.nc.sync.drain()
    wait_clock.add_sem_waits(
        drain_inst.ins, _tile_mod.ScopedClock({None: tick_clock.global_clock})
    )
    si = drain_inst.ins.sync_info
    waits = list(si.on_wait) if si is not None else []
    if len(waits) > 1:
        si.on_wait = waits[:1]
        for w in waits[1:]:
            d2 = self.nc.sync.drain()
            d2.ins.sync_info = mybir.SyncInfo(on_wait=[w], on_update=[])
    self.nc.all_engine_barrier()
    assert self.sems is not None
    popped = self.nc._tile_sem_poison_stack.pop()
    assert popped is self._sem_poison
    self.nc.clear_and_free_semaphores(list(self.sems.allocated().values()))
    self.nc.all_engine_barrier()


_tile_mod.TileContext._drain_and_barrier = _split_drain_and_barrier

F32 = mybir.dt.float32
F16 = mybir.dt.float16
AL = mybir.AluOpType
AFT = mybir.ActivationFunctionType

B = 4096
N_CORES = 8
B_LOC = B // N_CORES          # 512 samples per core
P = 128                       # partitions (samples per block)
NBLK = B_LOC // P             # 4 blocks
NA = 2048                     # max atoms
W = 3 * NA                    # 6144 floats per sample
NEWTON_ITERS = 8
EPS = 1e-12

NTMP = 64


def build_bass(n_repeat=1):
    nc = bass.Bass("TRN2", target_bir_lowering=False, debug=False)

    x_d = nc.dram_tensor("x", [B_LOC, W], F32, kind="ExternalInput")
    y_d = nc.dram_tensor("y", [B_LOC, W], F32, kind="ExternalInput")
    # consts: cols [0, W) = iota3, [W, W+4) = n_valid, [W+4, W+8) = 1/n_valid
    consts_d = nc.dram_tensor("consts", [P, W + 2 * NBLK], F32, kind="ExternalInput")
    out_d = nc.dram_tensor("out", [P, NBLK], F32, kind="ExternalOutput")

    with TileContext(nc) as tc:
        with (
            tc.tile_pool(name="const", bufs=1) as pconst,
            tc.tile_pool(name="px", bufs=2) as px,
            tc.tile_pool(name="py", bufs=2) as py,
            tc.tile_pool(name="pxm", bufs=2) as pxm,
            tc.tile_pool(name="pym", bufs=2) as pym,
            tc.tile_pool(name="pscr", bufs=1) as pscr,
            tc.tile_pool(name="pa", bufs=3) as ppa,
            tc.tile_pool(name="pstat", bufs=1) as pstat,
        ):
            consts_t = pconst.tile([P, W + 2 * NBLK], F32)
            nc.gpsimd.dma_start(consts_t[:, :], consts_d[:, :])
            iota_t = consts_t[:, 0:W]
            nv_t = consts_t[:, W : W + NBLK]
            invn_t = consts_t[:, W + NBLK : W + 2 * NBLK]

            # accumulator tiles per writing engine (no false sharing)
            stats_m = pstat.tile([P, 5 * NBLK], F32)    # DVE: moments q=4..8
            # ACT: Sx (0-2), Sy (3-5), Qx (6), Qy (7), moments q=0..3 (8-11)
            stats_a = pstat.tile([P, 12 * NBLK], F32)
            tmp = pstat.tile([P, NTMP * NBLK], F32)
            dummy = pstat.tile([P, 64], F32)            # DVE-side absorbers
            adummy = pstat.tile([P, 64], F32)           # ACT-side absorbers
            pd = pscr.tile([P, NA], F16)                # DVE-reduced products
            qscr = pscr.tile([P, W], F16)               # ACT activation out
            sscr = pscr.tile([P, NA], F16)

            # ---- explicit-order plumbing -------------------------------
            last_dve = [None]
            last_act = [None]
            touch_idx = [0]
            adummy_idx = [0]

            def order(bi, prev_list):
                if prev_list[0] is not None:
                    add_dep_helper(bi.ins, prev_list[0].ins, sync=False,
                                   reason="wait-funnel order")
                prev_list[0] = bi
                return bi

            def dve(bi):
                return order(bi, last_dve)

            def act(bi):
                return order(bi, last_act)

            def touch(ap):
                # tiny DVE read: absorbs one cross-proc wait into a dedicated
                # instruction so the real consumers carry at most one wait
                k = touch_idx[0] % 64
                touch_idx[0] += 1
                return dve(nc.vector.tensor_copy(dummy[:, k : k + 1], ap))

            def touch_reduce(ap):
                k = touch_idx[0] % 64
                touch_idx[0] += 1
                return dve(nc.vector.tensor_reduce(
                    dummy[:, k : k + 1], ap, axis=mybir.AxisListType.X,
                    op=AL.max,
                ))

            def act_touch(ap):
                k = adummy_idx[0] % 64
                adummy_idx[0] += 1
                return act(nc.scalar.activation(
                    adummy[:, k : k + 1], ap, AFT.Copy,
                ))

            touch(consts_t[:, :1])

            def slot_m(q, b):
                return stats_m[:, q * NBLK + b : q * NBLK + b + 1]

            def slot_a(q, b):
                return stats_a[:, q * NBLK + b : q * NBLK + b + 1]

            sa_r = stats_a[:, :].rearrange("p (q b) -> p b q", b=NBLK)

            # ACT-destined product q -> stats_a slot, DVE-destined -> stats_m
            A_PRODS = [0, 1, 2, 3]          # reduced on ACT (slots 8..11)
            D_PRODS = [4, 5, 6, 7, 8]       # reduced on DVE (slots 0..4)

            # per-(global block) x/y tiles, rotating through bufs=2 pools
            xts = {}
            yts = {}
            xms = {}
            yms = {}

            def dma_block(g, eng):
                b = g % NBLK
                xt = px.tile([P, W], F32)
                yt = py.tile([P, W], F32)
                xts[g], yts[g] = xt, yt
                if eng is None:
                    nc.sync.dma_start(xt[:, :], x_d[b * P : (b + 1) * P, :])
                    nc.gpsimd.dma_start(yt[:, :], y_d[b * P : (b + 1) * P, :])
                else:
                    act(nc.scalar.dma_start(xt[:, :], x_d[b * P : (b + 1) * P, :]))
                    act(nc.scalar.dma_start(yt[:, :], y_d[b * P : (b + 1) * P, :]))

            NG = n_repeat * NBLK
            dma_block(0, None)
            dma_block(1, None)

            for g in range(NG):
                r, b = divmod(g, NBLK)
                xt, yt = xts.pop(g), yts.pop(g)

                if g >= 1:
                    # absorb the ACT semaphore before touching block g-1's
                    # buffers: reduce over its 12 ACT accum slots so the
                    # wait covers whichever accum op was scheduled last
                    touch_reduce(sa_r[:, (g - 1) % NBLK, :])

                nvb = nv_t[:, b : b + 1]
                xm = pxm.tile([P, 3 * NA], F16)
                ym = pym.tile([P, 3 * NA], F16)
                xms[g], yms[g] = xm, ym
                # masked deinterleave: out plane-major fp16, in interleaved f32
                xv = xt[:, :].rearrange("p (a c) -> p a c", c=3)
                yv = yt[:, :].rearrange("p (a c) -> p a c", c=3)
                iv = iota_t.rearrange("p (a c) -> p a c", c=3)
                xmv = xm[:, :].rearrange("p (c a) -> p a c", a=NA)
                ymv = ym[:, :].rearrange("p (c a) -> p a c", a=NA)
                stt_x = dve(nc.vector.scalar_tensor_tensor(
                    out=xmv, in0=iv, scalar=nvb, in1=xv,
                    op0=AL.is_lt, op1=AL.mult,
                ))
                dve(nc.vector.scalar_tensor_tensor(
                    out=ymv, in0=iv, scalar=nvb, in1=yv,
                    op0=AL.is_lt, op1=AL.mult,
                ))

                # ACT absorber on xm only: lets next block's DMAs launch while
                # this block's ym mask is still being written
                act_touch(xm[:, 0:1])
                if g + 1 < NG:
                    # absorb the DVE tick of STT-ym(g-1): the g+1 DMAs
                    # overwrite the x/y buffers block g-1 read, so with this
                    # covered they carry only their WAW DMA-sem wait
                    if g >= 1 and (g - 1) in yms:
                        act_touch(yms[g - 1][:, 0:1])
                    dma_block(g + 1, "act")

                def plane(t, i):
                    return t[:, i * NA : (i + 1) * NA]

                # DVE: first 3 ACT-destined products (fp16 packed TT, 2x mode)
                pas = []
                for q in A_PRODS[:3]:
                    i, j = divmod(q, 3)
                    pa = ppa.tile([P, NA], F16)
                    pas.append(pa)
                    dve(nc.vector.tensor_tensor(
                        out=pa[:, :], in0=plane(xm, i), in1=plane(ym, j),
                        op=AL.mult))

                # ACT: Qx (square-accumulate whole row); absorb the ym mask
                # tick before Qy so it carries at most its qscr WAW wait
                act(nc.scalar.activation(
                    qscr[:, :], xm[:, :], AFT.Square, accum_out=slot_a(6, b)))
                act_touch(ym[:, 0:1])
                act(nc.scalar.activation(
                    qscr[:, :], ym[:, :], AFT.Square, accum_out=slot_a(7, b)))
                # absorb the DVE tick after the 3rd product, then accumulate
                act_touch(pas[2][:, 0:1])
                for k, q in enumerate(A_PRODS[:3]):
                    act(nc.scalar.activation(
                        sscr[:, :], pas[k][:, :], AFT.Copy,
                        accum_out=slot_a(8 + q, b)))
                for i in range(3):
                    act(nc.scalar.activation(
                        sscr[:, :], plane(xm, i), AFT.Copy,
                        accum_out=slot_a(i, b)))
                    act(nc.scalar.activation(
                        sscr[:, :], plane(ym, i), AFT.Copy,
                        accum_out=slot_a(3 + i, b)))

                # DVE: 5 products reduced in place on DVE
                for q in D_PRODS:
                    i, j = divmod(q, 3)
                    dve(nc.vector.tensor_tensor(
                        out=pd[:, :], in0=plane(xm, i), in1=plane(ym, j),
                        op=AL.mult))
                    dve(nc.vector.tensor_reduce(
                        slot_m(q - 4, b), pd[:, :],
                        axis=mybir.AxisListType.X, op=AL.add))

                # last ACT-destined product: reuses pa buffer 0; absorb the
                # WAR on that buffer's ACT accum into a DVE touch so the TT
                # carries at most its same-engine WAW wait
                q = A_PRODS[3]
                i, j = divmod(q, 3)
                touch(slot_a(8 + A_PRODS[0], b))
                pa = ppa.tile([P, NA], F16)
                dve(nc.vector.tensor_tensor(
                    out=pa[:, :], in0=plane(xm, i), in1=plane(ym, j),
                    op=AL.mult))
                act_touch(pa[:, 0:1])
                act(nc.scalar.activation(
                    sscr[:, :], pa[:, :], AFT.Copy,
                    accum_out=slot_a(8 + q, b)))

                if b == NBLK - 1:
                    _tail(nc, dve, act, touch, touch_reduce, act_touch,
                          stats_m, stats_a, sa_r, tmp, nv_t, invn_t, out_d)

    _strip_redundant_waits(nc)
    return nc


def _strip_redundant_waits(nc):
    """Drop sem waits that are provably redundant but that Tile's clock
    elision missed; walrus accepts at most ONE wait per instruction.

    1. Prefetch DMAs (writing xt_*/yt_* SBUF tiles): their WAW vs the
       previous DMA into the same buffer is implied by the reader-WAR
       engine wait (the DVE mask op only read the buffer after that DMA's
       completion sem) -> keep only the engine (reader) wait.
    2. The out DMA (DRAM "out"): its WAW vs the previous pass's out DMA is
       implied by the RAW on r0 (the tail could only overwrite r0 after the
       previous out DMA's read) -> keep only the DVE wait.
    """
    for blk in nc.m.functions[0].blocks:
        for ins in blk.instructions:
            if type(ins).__name__ != "InstDMACopy":
                continue
            si = getattr(ins, "sync_info", None)
            if si is None or si.on_wait is None or len(si.on_wait) <= 1:
                continue
            name = str(ins.outs[0].memref)
            waits = list(si.on_wait)
            eng = [w for w in waits if w.ant_name.startswith(("DVE", "Activation", "Pool", "SP", "PE"))]
            dma = [w for w in waits if w.ant_name.startswith("DMA")]
            if name.startswith(("xt_", "yt_", "out")) and len(eng) == 1 and dma:
                si.on_wait = eng
            assert len(si.on_wait) <= 1, (
                f"unresolved multi-wait DMA {ins.name} -> {name}: "
                f"{[w.ant_name for w in waits]}"
            )


def _tail(nc, dve, act, touch, touch_reduce, act_touch,
          stats_m, stats_a, sa_r, tmp, nv_t, invn_t, out_d):
    # absorb the final ACT ticks before the finishing math reads stats_a
    touch_reduce(stats_a[:, :])

    def S(q):
        # moments q=0..3: ACT accums -> stats_a slots 8..11
        # moments q=4..8: DVE reduces -> stats_m slots 0..4
        # q=9..11: Sx; 12..14: Sy; 15: Qx; 16: Qy  (stats_a 0..7)
        if q < 4:
            return stats_a[:, (8 + q) * NBLK : (9 + q) * NBLK]
        if q < 9:
            return stats_m[:, (q - 4) * NBLK : (q - 3) * NBLK]
        qa = q - 9
        return stats_a[:, qa * NBLK : (qa + 1) * NBLK]

    tmp_idx = [0]

    def T():
        k = tmp_idx[0]
        tmp_idx[0] += 1
        assert k < NTMP
        return tmp[:, k * NBLK : (k + 1) * NBLK]

    def MUL(o, a, c):
        dve(nc.vector.tensor_tensor(out=o, in0=a, in1=c, op=AL.mult))

    def ADD(o, a, c):
        dve(nc.vector.tensor_tensor(out=o, in0=a, in1=c, op=AL.add))

    def SUB(o, a, c):
        dve(nc.vector.tensor_tensor(out=o, in0=a, in1=c, op=AL.subtract))

    def SMUL(o, a, c):
        dve(nc.vector.tensor_scalar_mul(o, a, float(c)))

    def SADD(o, a, c):
        dve(nc.vector.tensor_scalar_add(o, a, float(c)))

    def SMAX(o, a, c):
        dve(nc.vector.tensor_scalar_max(o, a, float(c)))

    invn4 = invn_t
    scr = T()

    # u_i = Sx_i * invn ; C_ij = M_ij - u_i * Sy_j  (C overwrites M slots)
    u = [T() for _ in range(3)]
    for i in range(3):
        MUL(u[i], S(9 + i), invn4)
    for i in range(3):
        for j in range(3):
            MUL(scr, u[i], S(12 + j))
            SUB(S(3 * i + j), S(3 * i + j), scr)

    # ssq = Qx + Qy - (|Sx|^2 + |Sy|^2) * invn
    ssq = T()
    ADD(ssq, S(15), S(16))
    acc = T()
    MUL(scr, u[0], S(9))
    MUL(acc, u[1], S(10))
    ADD(acc, acc, scr)
    MUL(scr, u[2], S(11))
    ADD(acc, acc, scr)
    SUB(ssq, ssq, acc)
    sy2 = T()
    MUL(sy2, S(12), S(12))
    MUL(scr, S(13), S(13))
    ADD(sy2, sy2, scr)
    MUL(scr, S(14), S(14))
    ADD(sy2, sy2, scr)
    MUL(sy2, sy2, invn4)
    SUB(ssq, ssq, sy2)

    Sxx, Sxy, Sxz = S(0), S(1), S(2)
    Syx, Syy, Syz = S(3), S(4), S(5)
    Szx, Szy, Szz = S(6), S(7), S(8)

    # squares of all 9 C entries
    sq = [T() for _ in range(9)]
    for q in range(9):
        MUL(sq[q], S(q), S(q))
    (Sxx2, Sxy2, Sxz2, Syx2, Syy2, Syz2, Szx2, Szy2, Szz2) = sq

    # C2 = -2 * sum(C_ij^2)
    C2 = T()
    ADD(C2, Sxx2, Sxy2)
    for t in (Sxz2, Syx2, Syy2, Syz2, Szx2, Szy2, Szz2):
        ADD(C2, C2, t)
    SMUL(C2, C2, -2.0)

    # start the sqrt for the improved Newton seed early: it runs
    # on ACT while DVE builds C1/C0
    lamf = T()
    SMUL(lamf, C2, -1.5)
    SMAX(lamf, lamf, 0.0)
    act_touch(lamf[:, 0:1])
    act(nc.scalar.activation(lamf, lamf, AFT.Sqrt))

    # C1 = -8 * det(C); minor0 = Syy*Szz - Syz*Szy reused for E below
    minor0 = T()
    MUL(minor0, Syy, Szz)
    MUL(scr, Syz, Szy)
    SUB(minor0, minor0, scr)
    det = T()
    MUL(det, Sxx, minor0)
    m1 = T()
    MUL(m1, Syx, Szz)
    MUL(scr, Syz, Szx)
    SUB(m1, m1, scr)
    MUL(m1, Sxy, m1)
    SUB(det, det, m1)
    MUL(m1, Syx, Szy)
    MUL(scr, Syy, Szx)
    SUB(m1, m1, scr)
    MUL(m1, Sxz, m1)
    ADD(det, det, m1)
    C1 = T()
    SMUL(C1, det, -8.0)

    # C0 (Theobald's expansion)
    E = T()
    SMUL(E, minor0, -2.0)  # 2*(Syz*Szy - Syy*Szz)
    D = T()
    ADD(D, Syy2, Szz2)
    SUB(D, D, Sxx2)
    ADD(D, D, Syz2)
    ADD(D, D, Szy2)
    Fq = T()
    ADD(Fq, Sxy2, Sxz2)
    SUB(Fq, Fq, Syx2)
    SUB(Fq, Fq, Szx2)

    C0 = T()
    MUL(C0, Fq, Fq)
    a = T()
    bq = T()
    ADD(a, D, E)
    SUB(bq, D, E)
    MUL(a, a, bq)
    ADD(C0, C0, a)

    SxzpSzx = T(); ADD(SxzpSzx, Sxz, Szx)
    SyzpSzy = T(); ADD(SyzpSzy, Syz, Szy)
    SxypSyx = T(); ADD(SxypSyx, Sxy, Syx)
    SyzmSzy = T(); SUB(SyzmSzy, Syz, Szy)
    SxzmSzx = T(); SUB(SxzmSzx, Sxz, Szx)
    SxymSyx = T(); SUB(SxymSyx, Sxy, Syx)
    SxxpSyy = T(); ADD(SxxpSyy, Sxx, Syy)
    SxxmSyy = T(); SUB(SxxmSyy, Sxx, Syy)
    pmm = T(); SUB(pmm, SxxmSyy, Szz)
    pmp = T(); ADD(pmp, SxxmSyy, Szz)
    ppm = T(); SUB(ppm, SxxpSyy, Szz)
    ppp = T(); ADD(ppp, SxxpSyy, Szz)

    L = T()
    R = T()
    # term3
    MUL(scr, SxzpSzx, SyzmSzy)
    MUL(L, SxymSyx, pmm)
    SUB(L, L, scr)
    MUL(scr, SxzmSzx, SyzpSzy)
    MUL(R, SxymSyx, pmp)
    SUB(R, R, scr)
    MUL(L, L, R)
    ADD(C0, C0, L)
    # term4 (both brackets negated; product keeps sign)
    MUL(scr, SxzpSzx, SyzpSzy)
    MUL(L, SxypSyx, ppm)
    ADD(L, L, scr)
    MUL(scr, SxzmSzx, SyzmSzy)
    MUL(R, SxypSyx, ppp)
    ADD(R, R, scr)
    MUL(L, L, R)
    ADD(C0, C0, L)
    # term5
    MUL(scr, SxypSyx, SyzpSzy)
    MUL(L, SxzpSzx, pmp)
    ADD(L, L, scr)
    MUL(scr, SxymSyx, SyzmSzy)
    MUL(R, SxzpSzx, ppp)
    SUB(R, R, scr)
    MUL(L, L, R)
    ADD(C0, C0, L)
    # term6
    MUL(scr, SxypSyx, SyzmSzy)
    MUL(L, SxzmSzx, pmm)
    ADD(L, L, scr)
    MUL(scr, SxymSyx, SyzpSzy)
    MUL(R, SxzmSzx, ppm)
    SUB(R, R, scr)
    MUL(L, L, R)
    ADD(C0, C0, L)

    # Newton seed: min of the two upper bounds ssq/2 and
    # sqrt(3)*||C||_F = sqrt(-1.5*C2) -- at most ~1.7x the root,
    # so 8 iterations reach the f32 noise floor
    lam = T()
    SMUL(lam, ssq, 0.5)
    touch(lamf[:, 0:1])
    dve(nc.vector.tensor_tensor(out=lam, in0=lam, in1=lamf, op=AL.min))
    twoC2 = T()
    SMUL(twoC2, C2, 2.0)
    t1 = T()
    Pv = T()
    cv = T()
    dv = T()
    rv = T()
    for _ in range(NEWTON_ITERS):
        MUL(t1, lam, lam)
        MUL(Pv, t1, t1)
        MUL(cv, C2, t1)
        ADD(Pv, Pv, cv)
        MUL(cv, C1, lam)
        ADD(Pv, Pv, cv)
        ADD(Pv, Pv, C0)
        SMUL(dv, t1, 4.0)
        ADD(dv, dv, twoC2)
        MUL(dv, dv, lam)
        ADD(dv, dv, C1)
        SADD(dv, dv, EPS)
        dve(nc.vector.reciprocal(rv, dv))
        MUL(cv, Pv, rv)
        SUB(lam, lam, cv)
        SMAX(lam, lam, 0.0)

    # rank-1 (n==2) override: lam = sqrt(-C2/2)
    lr1 = T()
    SMUL(lr1, C2, -0.5)
    SMAX(lr1, lr1, 0.0)
    act_touch(lr1[:, 0:1])
    act(nc.scalar.activation(lr1, lr1, AFT.Sqrt))
    wsel = T()
    dve(nc.vector.tensor_scalar(
        out=wsel, in0=nv_t, scalar1=2.0, scalar2=None,
        op0=AL.is_equal,
    ))
    # absorb the ACT->DVE wait for the sqrt result
    touch(lr1[:, 0:1])
    SUB(lr1, lr1, lam)
    MUL(lr1, wsel, lr1)
    ADD(lam, lam, lr1)

    # msd = max(ssq - 2 lam, 0) * invn; rmsd = sqrt(msd + eps)
    msd = T()
    SMUL(msd, lam, 2.0)
    SUB(msd, ssq, msd)
    SMAX(msd, msd, 0.0)
    MUL(msd, msd, invn4)
    SADD(msd, msd, EPS)
    r0 = T()
    act_touch(msd[:, 0:1])
    act(nc.scalar.activation(r0, msd, AFT.Sqrt))
    # one Newton refinement of the sqrt (ACT sqrt ULP budget is loose)
    rec = T()
    touch(r0[:, 0:1])
    dve(nc.vector.reciprocal(rec, r0))
    MUL(rec, msd, rec)
    ADD(r0, r0, rec)
    SMUL(r0, r0, 0.5)

    # out DMA from the gpsimd queue: carries the RAW wait on r0 (DVE).
    # On repeat passes it also gets a WAW wait vs the previous pass's out
    # DMA, which _strip_redundant_waits drops (provably ordered: pass r's
    # tail overwrote r0 only after the WAR on pass r-1's out DMA read).
    nc.gpsimd.dma_start(out_d[:, :], r0)


_NC_CACHE = None


def _get_nc():
    global _NC_CACHE
    if _NC_CACHE is None:
        _NC_CACHE = build_bass()
    return _NC_CACHE


def make_in_maps(inp, tgt, al):
    inp = np.ascontiguousarray(np.asarray(inp, dtype=np.float32))
    tgt = np.ascontiguousarray(np.asarray(tgt, dtype=np.float32))
    al = np.asarray(al, dtype=np.int32)
    nv = (al + 1).astype(np.float32)
    iota = np.broadcast_to(np.repeat(np.arange(NA, dtype=np.float32), 3), (P, W))
    in_maps = []
    for c in range(N_CORES):
        s = slice(c * B_LOC, (c + 1) * B_LOC)
        nv_c = nv[s].reshape(NBLK, P).T
        consts = np.concatenate([iota, nv_c, 1.0 / nv_c], axis=1)
        in_maps.append({
            "x": np.ascontiguousarray(inp[s]),
            "y": np.ascontiguousarray(tgt[s]),
            "consts": np.ascontiguousarray(consts.astype(np.float32)),
        })
    return in_maps


def gather_output(core_outs):
    return np.concatenate(
        [np.asarray(o).T.reshape(B_LOC) for o in core_outs]
    ).astype(np.float32)


def run(inputs, **spmd_kwargs):
    nc = _get_nc()
    in_maps = make_in_maps(inputs["input"], inputs["target"], inputs["angles_length"])
    res = run_bass_kernel_spmd(nc, in_maps, list(range(N_CORES)), **spmd_kwargs)
    return gather_output([res.results[c]["out"] for c in range(N_CORES)]), res


def _host_qcp(inp, tgt, al):
    """Validated numpy QCP fallback (same math as the device kernel)."""
    dt = np.float32
    bsz = inp.shape[0]
    x = np.asarray(inp, dt).reshape(bsz, NA, 3)
    y = np.asarray(tgt, dt).reshape(bsz, NA, 3)
    al = np.asarray(al)
    nv = (al + 1).astype(dt)
    m3 = (np.arange(NA)[None, :] < (al[:, None] + 1)).astype(dt)[..., None]
    inv_n = (dt(1.0) / nv).astype(dt)
    xm = x * m3
    ym = y * m3
    Sx = xm.sum(1, dtype=dt)
    Sy = ym.sum(1, dtype=dt)
    M = np.einsum("bni,bnj->bij", xm, y).astype(dt)
    Qx = (xm * xm).sum((1, 2), dtype=dt)
    Qy = (ym * ym).sum((1, 2), dtype=dt)
    C = M - Sx[:, :, None] * Sy[:, None, :] * inv_n[:, None, None]
    ssq = Qx + Qy - ((Sx * Sx).sum(1) + (Sy * Sy).sum(1)) * inv_n
    Sxx, Sxy, Sxz = C[:, 0, 0], C[:, 0, 1], C[:, 0, 2]
    Syx, Syy, Syz = C[:, 1, 0], C[:, 1, 1], C[:, 1, 2]
    Szx, Szy, Szz = C[:, 2, 0], C[:, 2, 1], C[:, 2, 2]
    sq = [v * v for v in (Sxx, Sxy, Sxz, Syx, Syy, Syz, Szx, Szy, Szz)]
    Sxx2, Sxy2, Sxz2, Syx2, Syy2, Syz2, Szx2, Szy2, Szz2 = sq
    E = dt(2.0) * (Syz * Szy - Syy * Szz)
    D = Syy2 + Szz2 - Sxx2 + Syz2 + Szy2
    C2 = dt(-2.0) * sum(sq)
    C1 = dt(8.0) * (Sxx * Syz * Szy + Syy * Szx * Sxz + Szz * Sxy * Syx
                    - Sxx * Syy * Szz - Syz * Szx * Sxy - Szy * Syx * Sxz)
    SxzpSzx = Sxz + Szx; SyzpSzy = Syz + Szy; SxypSyx = Sxy + Syx
    SyzmSzy = Syz - Szy; SxzmSzx = Sxz - Szx; SxymSyx = Sxy - Syx
    SxxpSyy = Sxx + Syy; SxxmSyy = Sxx - Syy
    F = Sxy2 + Sxz2 - Syx2 - Szx2
    C0 = (F * F + (D + E) * (D - E)
          + (-(SxzpSzx) * SyzmSzy + SxymSyx * (SxxmSyy - Szz))
          * (-(SxzmSzx) * SyzpSzy + SxymSyx * (SxxmSyy + Szz))
          + (-(SxzpSzx) * SyzpSzy - SxypSyx * (SxxpSyy - Szz))
          * (-(SxzmSzx) * SyzmSzy - SxypSyx * (SxxpSyy + Szz))
          + (SxypSyx * SyzpSzy + SxzpSzx * (SxxmSyy + Szz))
          * (-(SxymSyx) * SyzmSzy + SxzpSzx * (SxxpSyy + Szz))
          + (SxypSyx * SyzmSzy + SxzmSzx * (SxxmSyy - Szz))
          * (-(SxymSyx) * SyzpSzy + SxzmSzx * (SxxpSyy - Szz)))
    lam = np.minimum(ssq * dt(0.5), np.sqrt(np.maximum(C2 * dt(-1.5), dt(0.0))))
    twoC2 = dt(2.0) * C2
    for _ in range(NEWTON_ITERS):
        t1 = lam * lam
        Pv = t1 * t1 + C2 * t1 + C1 * lam + C0
        dP = lam * (dt(4.0) * t1 + twoC2) + C1 + dt(1e-12)
        lam = np.maximum(lam - Pv / dP, dt(0.0))
    lam_r1 = np.sqrt(np.maximum(C2 * dt(-0.5), dt(0.0)))
    w = (nv == dt(2.0)).astype(dt)
    lam = lam + w * (lam_r1 - lam)
    msd = np.maximum(ssq - dt(2.0) * lam, dt(0.0)) * inv_n
    return np.sqrt(msd + dt(1e-12)).astype(np.float32)


def kernel(**inputs):
    try:
        return run(inputs)[0]
    except Exception as e:
        sys.stderr.write(f"kernel: device path failed ({type(e).__name__}); "
                         f"using host fallback\n")
        return _host_qcp(inputs["input"], inputs["target"],
                         inputs["angles_length"])
